# revision 7
# baseline (speedup 1.0000x reference)
"""Trainium2 Bass kernel for nn_CustomLoss_19061064859882.

loss = CE(y_pred, y_true) - penalty/N, where the penalty uses
p1 = softmax(y_pred)[:, 0] and per-class weights from the label histogram.

Device/host split: everything that is O(N*C) transcendental work — the
per-row logsumexp over the 128 classes — runs on the 8 NeuronCores
(data-parallel over rows, fp16 on the wire, exp on ScalarE + row-reduce on
VectorE). The remaining O(N) bookkeeping (picked-logit gather, label
bincount, per-class weighted sums, final scalar) is cheap vectorized numpy
on the host, done in float64:

    lse_i   = log(sum_c exp(y_pred[i, c]))          # device
    CE      = -(sum_i y_pred[i, y_i] - sum_i lse_i)/N
    p1_i    = exp(y_pred[i, 0] - lse_i)
    v_i     = y_i==0 ? ALPHA*log(p1+eps) : s[y_i]*log(1-p1+eps)
    loss    = CE - sum_i v_i / N

Per core: 32768 rows -> 8 batches of 4096 rows, rows packed 32 per
partition (fully linear 1MiB DMAs). Per batch only 4 instructions:
dma_in -> exp(ACT) -> reduce(DVE) -> ln(ACT into a persistent output
buffer). One 128KiB DMA out at the end.
"""

import sys

import numpy as np

if "/opt/trn_rl_repo" not in sys.path:
    sys.path.insert(0, "/opt/trn_rl_repo")

N_CORES = 8
N = 262144
C = 128  # classes
M = N // N_CORES  # rows per core
P = 128  # SBUF partitions
KB = 32  # rows per partition per batch
BATCH_ROWS = P * KB  # 4096
NB = M // BATCH_ROWS  # 8 batches per core
ALPHA = 0.5
BETA = 0.5
EPS = 1e-9

_CACHE: dict = {}


def _build_nc():
    import concourse.bacc as bacc
    import concourse.mybir as mybir
    import concourse.tile as tile

    f16 = mybir.dt.float16
    f32 = mybir.dt.float32
    Ln = mybir.ActivationFunctionType.Ln
    Exp = mybir.ActivationFunctionType.Exp

    nc = bacc.Bacc(
        "TRN2", target_bir_lowering=False, debug=False, num_devices=N_CORES
    )

    # Exp and Ln live in different default table-sets, so bacc would emit an
    # ACT_TABLE_LOAD (~2.7us) at every Exp<->Ln transition. Strip them from
    # every set except the one that holds both, so a single load serves the
    # whole kernel. (get_activation_tables is functools.cache'd; mutating the
    # returned sets is how we reach bacc's insert_act_table_loads pass.)
    import concourse.hw_specs as hw_specs

    tabs = hw_specs.get_activation_tables(nc.m.arch)
    if "natural_log_exp_and_others" in tabs:
        for name, funcs in tabs.items():
            if name != "natural_log_exp_and_others":
                funcs.discard(Exp)
                funcs.discard(Ln)

    y = nc.dram_tensor("y_pred", [M, C], f16, kind="ExternalInput").ap()
    out = nc.dram_tensor("out", [P, NB, KB], f32, kind="ExternalOutput").ap()

    # row(b, p, k) = b*4096 + p*32 + k  (8KB contiguous per partition/batch)
    y4 = y.rearrange("(b p k) c -> b p k c", b=NB, p=P, k=KB)

    with tile.TileContext(nc) as tc:
        with (
            tc.tile_pool(name="persist", bufs=1) as persist,
            tc.tile_pool(name="work", bufs=3) as work,
        ):
            obuf = persist.tile([P, NB, KB], f32)
            for b in range(NB):
                T = work.tile([P, KB, C], f16)
                nc.sync.dma_start(T[:], y4[b])
                E = work.tile([P, KB, C], f16)
                nc.scalar.activation(E[:], T[:], Exp)
                # Pairwise halving (fp16 TT hits the DVE 2x mode), then the
                # fp16 X-reduce runs on half the elements.
                H = work.tile([P, KB, C // 2], f16)
                nc.vector.tensor_add(H[:], E[:, :, 0 : C // 2], E[:, :, C // 2 : C])
                se = work.tile([P, KB], f32)
                nc.vector.reduce_sum(se[:], H[:], axis=mybir.AxisListType.X)
                nc.scalar.activation(obuf[:, b, :], se[:], Ln)
            nc.sync.dma_start(out[:], obuf[:])

    nc.finalize()
    return nc


def _get_nc():
    if "nc" not in _CACHE:
        _CACHE["nc"] = _build_nc()
    return _CACHE["nc"]


def _make_in_maps(y_pred: np.ndarray):
    y16 = np.asarray(y_pred).astype(np.float16)
    return [{"y_pred": np.ascontiguousarray(y16[c * M : (c + 1) * M])} for c in range(N_CORES)]


def _run(in_maps, trace=False, **kwargs):
    from concourse.bass_utils import run_bass_kernel_spmd

    nc = _get_nc()
    return run_bass_kernel_spmd(
        nc, in_maps, list(range(N_CORES)), trace=trace, **kwargs
    )


def _combine(results, y_pred: np.ndarray, y_true: np.ndarray) -> np.ndarray:
    yp = np.asarray(y_pred)
    yt = np.asarray(y_true).reshape(-1).astype(np.int64)

    # Per-row logsumexp from the device: out[p, b, k] is row b*4096 + p*32 + k.
    lse = np.empty(N, dtype=np.float64)
    for c in range(N_CORES):
        o = results[c]["out"].astype(np.float64)  # [P, NB, KB]
        lse[c * M : (c + 1) * M] = o.transpose(1, 0, 2).reshape(M)

    picked = np.take_along_axis(yp, yt[:, None], axis=1).reshape(-1).astype(np.float64)
    ce = -(picked.sum() - lse.sum()) / N

    p1 = np.exp(yp[:, 0].astype(np.float64) - lse)
    lp = np.log(p1 + EPS)
    lq = np.log((1.0 + EPS) - p1)
    nj = np.bincount(yt, minlength=C).astype(np.float64)
    s = BETA * (1.0 - nj / (N - nj[0]))
    v = np.where(yt == 0, ALPHA * lp, s[yt] * lq)
    loss = ce - v.sum() / N
    return np.asarray(loss, dtype=np.float32)


def kernel(y_pred: np.ndarray, y_true: np.ndarray) -> np.ndarray:
    in_maps = _make_in_maps(y_pred)
    res = _run(in_maps, trace=False)
    return _combine(res.results, y_pred, y_true)


# revision 16
# speedup vs baseline: 1.1000x; 1.1000x over previous
"""Trainium2 Bass kernel for nn_CustomLoss_19061064859882.

loss = CE(y_pred, y_true) - penalty/N, where the penalty uses
p1 = softmax(y_pred)[:, 0] and per-class weights from the label histogram.

Device/host split: everything that is O(N*C) transcendental work — the
per-row logsumexp over the 128 classes — runs on the 8 NeuronCores
(data-parallel over rows, fp16 on the wire, exp on ScalarE + row-reduce on
VectorE). The remaining O(N) bookkeeping (picked-logit gather, label
bincount, per-class weighted sums, final scalar) is cheap vectorized numpy
on the host, done in float64:

    lse_i   = log(sum_c exp(y_pred[i, c]))          # device
    CE      = -(sum_i y_pred[i, y_i] - sum_i lse_i)/N
    p1_i    = exp(y_pred[i, 0] - lse_i)
    v_i     = y_i==0 ? ALPHA*log(p1+eps) : s[y_i]*log(1-p1+eps)
    loss    = CE - sum_i v_i / N

Per core: 32768 rows -> 8 batches of 4096 rows, rows packed 32 per
partition (fully linear 1MiB DMAs). Per batch only 4 instructions:
dma_in -> exp(ACT) -> reduce(DVE) -> ln(ACT into a persistent output
buffer). One 128KiB DMA out at the end.
"""

import sys

import numpy as np

if "/opt/trn_rl_repo" not in sys.path:
    sys.path.insert(0, "/opt/trn_rl_repo")

N_CORES = 8
N = 262144
C = 128  # classes
M = N // N_CORES  # rows per core
P = 128  # SBUF partitions
KB = 32  # rows per partition per batch
BATCH_ROWS = P * KB  # 4096
NB = M // BATCH_ROWS  # 8 batches per core
ALPHA = 0.5
BETA = 0.5
EPS = 1e-9

# Per-core job list: (row_base, rows_per_partition). Big 4096-row jobs carry
# most of the work with 8KB-contiguous DMA descriptors; the last 4096 rows
# run as four 1024-row jobs so the trailing exp->add->reduce chain is short.
JOBS = [(b * 4096, 32) for b in range(7)] + [
    (7 * 4096 + t * 1024, 8) for t in range(4)
]

_CACHE: dict = {}


def _build_nc():
    import concourse.bacc as bacc
    import concourse.mybir as mybir
    import concourse.tile as tile

    f16 = mybir.dt.float16
    f32 = mybir.dt.float32
    Ln = mybir.ActivationFunctionType.Ln
    Exp = mybir.ActivationFunctionType.Exp

    nc = bacc.Bacc(
        "TRN2", target_bir_lowering=False, debug=False, num_devices=N_CORES
    )

    # Exp and Ln live in different default table-sets, so bacc would emit an
    # ACT_TABLE_LOAD (~2.7us) at every Exp<->Ln transition. Strip them from
    # every set except the one that holds both, so a single load serves the
    # whole kernel. (get_activation_tables is functools.cache'd; mutating the
    # returned sets is how we reach bacc's insert_act_table_loads pass.)
    import concourse.hw_specs as hw_specs

    tabs = hw_specs.get_activation_tables(nc.m.arch)
    if "natural_log_exp_and_others" in tabs:
        for name, funcs in tabs.items():
            if name != "natural_log_exp_and_others":
                funcs.discard(Exp)
                funcs.discard(Ln)

    y = nc.dram_tensor("y_pred", [M, C], f16, kind="ExternalInput").ap()
    out = nc.dram_tensor("out", [P, M // P], f32, kind="ExternalOutput").ap()

    with tile.TileContext(nc) as tc:
        with (
            tc.tile_pool(name="persist", bufs=1) as persist,
            tc.tile_pool(name="work", bufs=3) as work,
        ):
            obuf = persist.tile([P, M // P], f32)
            col = 0
            for base, kb in JOBS:
                # rows [base, base + P*kb): row = base + p*kb + k, so each
                # partition gets kb*C*2B contiguous bytes (8KB at kb=32).
                yj = y[base : base + P * kb].rearrange("(p k) c -> p k c", p=P)
                T = work.tile([P, kb, C], f16)
                nc.sync.dma_start(T[:], yj)
                E = work.tile([P, kb, C], f16)
                nc.scalar.activation(E[:], T[:], Exp)
                # Pairwise halving on GpSimd (otherwise idle), then the
                # fp16 X-reduce on Vector runs on half the elements; log
                # of the row-sums happens on the host.
                H = work.tile([P, kb, C // 2], f16)
                nc.gpsimd.tensor_add(H[:], E[:, :, 0 : C // 2], E[:, :, C // 2 : C])
                nc.vector.reduce_sum(
                    obuf[:, col : col + kb], H[:], axis=mybir.AxisListType.X
                )
                col += kb
            nc.sync.dma_start(out[:], obuf[:])

    nc.finalize()
    return nc


def _get_nc():
    if "nc" not in _CACHE:
        _CACHE["nc"] = _build_nc()
    return _CACHE["nc"]


def _make_in_maps(y_pred: np.ndarray):
    y16 = np.asarray(y_pred).astype(np.float16)
    return [{"y_pred": np.ascontiguousarray(y16[c * M : (c + 1) * M])} for c in range(N_CORES)]


def _run(in_maps, trace=False, **kwargs):
    from concourse.bass_utils import run_bass_kernel_spmd

    nc = _get_nc()
    return run_bass_kernel_spmd(
        nc, in_maps, list(range(N_CORES)), trace=trace, **kwargs
    )


def _combine(results, y_pred: np.ndarray, y_true: np.ndarray) -> np.ndarray:
    yp = np.asarray(y_pred)
    yt = np.asarray(y_true).reshape(-1).astype(np.int64)

    # Per-row sumexp from the device: out[p, col] with col layout per JOBS.
    rowmap = np.empty((P, M // P), dtype=np.int64)
    col = 0
    for base, kb in JOBS:
        rowmap[:, col : col + kb] = (
            base + np.arange(P)[:, None] * kb + np.arange(kb)[None, :]
        )
        col += kb
    lse = np.empty(N, dtype=np.float64)
    for c in range(N_CORES):
        o = np.log(results[c]["out"].astype(np.float64))  # [P, M // P]
        lse[c * M + rowmap.reshape(-1)] = o.reshape(-1)

    picked = np.take_along_axis(yp, yt[:, None], axis=1).reshape(-1).astype(np.float64)
    ce = -(picked.sum() - lse.sum()) / N

    p1 = np.exp(yp[:, 0].astype(np.float64) - lse)
    lp = np.log(p1 + EPS)
    lq = np.log((1.0 + EPS) - p1)
    nj = np.bincount(yt, minlength=C).astype(np.float64)
    s = BETA * (1.0 - nj / (N - nj[0]))
    v = np.where(yt == 0, ALPHA * lp, s[yt] * lq)
    loss = ce - v.sum() / N
    return np.asarray(loss, dtype=np.float32)


def kernel(y_pred: np.ndarray, y_true: np.ndarray) -> np.ndarray:
    in_maps = _make_in_maps(y_pred)
    res = _run(in_maps, trace=False)
    return _combine(res.results, y_pred, y_true)
